# revision 1
# baseline (speedup 1.0000x reference)
"""BoundaryAwareSmoothAttention Trainium2 kernel.

Math (per batch b, full image HW=4096, C=64):
  Q = Wq x, K = Wk x, V = Wv x                  (1x1 convs, biases are zero)
  S[n,m] = q_n . k_m                            (energy)
  edge[m] = sigmoid(We2 . relu(BN(conv3x3(x))) + be2)
  mod[m]  = 1 + beta*edge[m]
  fa[n,m] = exp(S[n,m]) * mod[m]                (softmax Z cancels in L1 renorm)
  out[c,n] = gamma * (sum_m V[c,m] fa[n,m]) / (sum_m fa[n,m]) + x[c,n]

Sharding: 8 cores = 4 batches x 2 query-halves (n in [h*2048, h*2048+2048)).

Layout: key-dim m lives on SBUF partitions (S^T tiles), so no transposes are
needed anywhere.  The edge modulation is folded into the AV stationary weights
(V'[c,m] = V[c,m]*mod[m]) and the L1 denominator comes free as a mod[m] column
appended to each V'^T chunk, so the exp is a pure exp(S - 32) with a constant
bias and the 64-instruction ACT exp stream starts as soon as the first QK
matmul lands.  All matmuls run in float32r (TF32-like, full PE rate).
"""

import numpy as np

import concourse.bass as bass
import concourse.tile as tile
from concourse import bacc, mybir
from concourse.bass_utils import run_bass_kernel_spmd

F32 = mybir.dt.float32
F32R = mybir.dt.float32r
AF = mybir.ActivationFunctionType
ALU = mybir.AluOpType

C = 64
CH = 32
HW = 4096
NQ = 2048  # queries per core
QB = 1024  # q-block (exp instruction width)
N_CORES = 8
SHIFT = 32.0
BN_EPS = 1e-5

# scheduling knobs (tuned via TimelineSim sweep)
CFG = {
    "ab_mode": "pergroup",   # pergroup | dma | bulk0
    "sigma": "act",           # poly | act
    "proj_dup": "dma",        # dma only: f32r matmuls must write psum partition 0
    "bg_skips": (3, 6, 9),
    "bg_start": 1,
    "lag_hi": 4, "lag_switch": 13, "lag_lo": 2,
    "fa_bufs": 10,
}


def build_program(beta: float, gamma: float, be2: float):
    nc = bacc.Bacc("TRN2", target_bir_lowering=False, debug=False,
                   num_devices=N_CORES)

    def din(name, shape):
        return nc.dram_tensor(name, shape, F32, kind="ExternalInput").ap()

    x_d = din("x", [C, HW])
    xq_d = din("xq", [C, NQ])
    wq_d = din("wq_t", [C, C])
    wk_d = din("wk_t", [C, C])
    wv_d = din("wv_t", [C, C])
    wcatA_d = din("wcatA", [128, 96])
    wcatB_d = din("wcatB", [64, 96])
    we2_d = din("we2_t", [CH, 2])
    bnt_d = din("bn_t", [CH, 1])
    out_d = nc.dram_tensor("out", [C, NQ], F32, kind="ExternalOutput").ap()

    with tile.TileContext(nc) as tc:
        with (
            tc.tile_pool(name="consts", bufs=1) as consts,
            tc.tile_pool(name="bigs", bufs=1) as bigs,
            tc.tile_pool(name="fa_p", bufs=CFG["fa_bufs"]) as fa_p,
            tc.tile_pool(name="ep", bufs=2) as ep,
            tc.tile_pool(name="ps", bufs=2, space="PSUM") as ps,
            tc.tile_pool(name="ps_bg", bufs=2, space="PSUM") as ps_bg,
            tc.tile_pool(name="ps_o", bufs=1, space="PSUM") as ps_o,
        ):
            # ---- big SBUF tensors -----------------------------------------
            A = bigs.tile([128, HW], F32R)      # [x_m1 ; x]
            B = bigs.tile([64, HW], F32R)       # x_p1
            x_r = bigs.tile([C, HW], F32R)      # x (K proj rhs / VT lhsT)
            xq_r = bigs.tile([C, NQ], F32R)
            xq_f = bigs.tile([C, NQ], F32)
            K2 = bigs.tile([128, HW], F32R)     # K duplicated on both halves
            Q2 = bigs.tile([128, NQ], F32R)     # Q duplicated on both halves
            VT = bigs.tile([128, 66 * 32], F32R)  # chunks [V\'^T | mod | 0]
            relu_sb = bigs.tile([CH, HW], F32R)

            # ---- constant tiles -------------------------------------------
            wq_r = consts.tile([C, C], F32R)
            wk_r = consts.tile([C, C], F32R)
            wv_r = consts.tile([128, C], F32R)  # rows 0-63 and 64-127
            wcatA_r = consts.tile([128, 96], F32R)
            wcatB_r = consts.tile([64, 96], F32R)
            we2_r = consts.tile([CH, 2], F32R)
            bnt_sb = consts.tile([CH, 1], F32)
            ones_t = consts.tile([65, 64], F32R)
            ones_f = consts.tile([65, 64], F32)
            z64 = consts.tile([64, 64], F32)
            z128 = consts.tile([128, 64], F32)
            b_shift = consts.tile([128, 1], F32)
            b_be2 = consts.tile([128, 1], F32)
            eg_sb = consts.tile([128, 64], F32)
            eg2_sb = consts.tile([128, 64], F32)
            modt = consts.tile([128, 64], F32)
            modr = consts.tile([128, 64], F32R)
            VT_v = VT[:].rearrange("p (j w) -> p j w", w=66)
            modr_v = modr[:].rearrange("p (j two) -> p j two", two=2)

            # small weights via HWDGE stages + DVE casts (fast head);
            # big x casting DMAs on the gpsimd queue, QK-path first.
            wk_f = consts.tile([C, C], F32)
            wq_f = consts.tile([C, C], F32)
            nc.sync.dma_start(out=wk_f[:], in_=wk_d[:])
            nc.sync.dma_start(out=wq_f[:], in_=wq_d[:])
            nc.vector.tensor_copy(wk_r[:], wk_f[:])
            nc.vector.tensor_copy(wq_r[:], wq_f[:])
            nc.gpsimd.dma_start(out=x_r[:, 0:1024], in_=x_d[:, 0:1024])
            nc.gpsimd.dma_start(out=xq_r[:, 0:1024], in_=xq_d[:, 0:1024])
            nc.gpsimd.dma_start(out=xq_r[:, 1024:NQ], in_=xq_d[:, 1024:NQ])
            nc.gpsimd.dma_start(out=x_r[:, 1024:HW], in_=x_d[:, 1024:HW])
            nc.gpsimd.dma_start(out=wcatA_r[:], in_=wcatA_d[:])
            nc.gpsimd.dma_start(out=wcatB_r[:], in_=wcatB_d[:])
            nc.gpsimd.dma_start(out=A[64:128, :], in_=x_d[:])
            if CFG["ab_mode"] == "dma":
                nc.gpsimd.dma_start(out=A[0:64, 1:HW], in_=x_d[:, 0:HW - 1])
                nc.gpsimd.dma_start(out=B[0:64, 0:HW - 1], in_=x_d[:, 1:HW])
            nc.gpsimd.dma_start(out=wv_r[0:64, :], in_=wv_d[:])
            nc.gpsimd.dma_start(out=wv_r[64:128, :], in_=wv_d[:])
            nc.gpsimd.dma_start(out=we2_r[:], in_=we2_d[:])

            nc.vector.memset(ones_f[:], 1.0)
            nc.vector.tensor_copy(ones_t[:], ones_f[:])
            nc.vector.memset(z64[:], 0.0)
            nc.vector.memset(z128[:], 0.0)
            nc.vector.memset(b_shift[:], -SHIFT)
            nc.vector.memset(b_be2[:], -be2)
            gam_f = consts.tile([65, 64], F32)
            gam_r = consts.tile([65, 64], F32R)
            nc.vector.memset(gam_f[:], gamma)
            nc.vector.tensor_copy(gam_r[:], gam_f[:])

            # ---- Q/K projection emitters (interleaved into the loop) ------
            def emit_qproj(t):
                # early tiles: col-tiled pair fills psum partitions 0-63 and
                # 64-127 (no DMA); later tiles ride the idle DMA queue
                if False:
                    qp = ps.tile([128, 512], F32, tag="ps", name=f"qp{t}")
                    nc.tensor.matmul(qp[0:64, :], wq_r[:],
                                     xq_r[:, 512 * t:512 * t + 512],
                                     start=True, stop=True)
                    nc.tensor.matmul(qp[64:128, :], wq_r[:],
                                     xq_r[:, 512 * t:512 * t + 512],
                                     start=True, stop=True,
                                     tile_position=(0, 64))
                    nc.vector.tensor_copy(Q2[:, 512 * t:512 * t + 512], qp[:])
                else:
                    qp = ps.tile([64, 512], F32, tag="ps", name=f"qp{t}")
                    nc.tensor.matmul(qp[:], wq_r[:],
                                     xq_r[:, 512 * t:512 * t + 512],
                                     start=True, stop=True)
                    nc.vector.tensor_copy(Q2[0:64, 512 * t:512 * t + 512], qp[:])
                    nc.sync.dma_start(out=Q2[64:128, 512 * t:512 * t + 512],
                                      in_=Q2[0:64, 512 * t:512 * t + 512])

            def emit_kproj(t):
                if False:
                    kp = ps.tile([128, 512], F32, tag="ps", name=f"kp{t}")
                    nc.tensor.matmul(kp[0:64, :], wk_r[:],
                                     x_r[:, 512 * t:512 * t + 512],
                                     start=True, stop=True)
                    nc.tensor.matmul(kp[64:128, :], wk_r[:],
                                     x_r[:, 512 * t:512 * t + 512],
                                     start=True, stop=True,
                                     tile_position=(0, 64))
                    nc.vector.tensor_copy(K2[:, 512 * t:512 * t + 512], kp[:])
                else:
                    kp = ps.tile([64, 512], F32, tag="ps", name=f"kp{t}")
                    nc.tensor.matmul(kp[:], wk_r[:],
                                     x_r[:, 512 * t:512 * t + 512],
                                     start=True, stop=True)
                    nc.vector.tensor_copy(K2[0:64, 512 * t:512 * t + 512], kp[:])
                    nc.sync.dma_start(out=K2[64:128, 512 * t:512 * t + 512],
                                      in_=K2[0:64, 512 * t:512 * t + 512])

            A_vw = A[0:64, :].rearrange("p (y x) -> p y x", x=64)
            B_vw = B[0:64, :].rearrange("p (y x) -> p y x", x=64)

            def emit_xprep():
                nc.vector.tensor_copy(VT_v[:, 0:32, 65], z128[:, 0:32])
                if CFG["ab_mode"] == "bulk0":
                    nc.vector.tensor_copy(A[0:64, 1:HW], x_r[:, 0:HW - 1])
                    nc.vector.tensor_copy(B[0:64, 0:HW - 1], x_r[:, 1:HW])
                if CFG["ab_mode"] in ("bulk0", "dma"):
                    nc.vector.tensor_copy(A_vw[:, :, 0], z64[:])
                    nc.vector.tensor_copy(B_vw[:, :, 63], z64[:])

            def bg_prep(t):
                if CFG["ab_mode"] != "pergroup":
                    return
                # build the x_m1 (A rows 0-63) and x_p1 (B) slices this and
                # the next conv tile will read, plus their SAME-pad zeros
                r0 = 0 if t == 0 else 512 * t + 576
                r1 = min(512 * t + 1088, HW)
                if r1 <= r0:
                    return
                a0 = max(r0, 1)
                nc.vector.tensor_copy(A[0:64, a0:r1], x_r[:, a0 - 1:r1 - 1])
                nc.vector.tensor_copy(B[0:64, r0:r1 - 1], x_r[:, r0 + 1:r1])
                y0, y1 = r0 // 64, r1 // 64
                nc.vector.tensor_copy(A_vw[:, y0:y1, 0], z64[:, 0:y1 - y0])
                nc.vector.tensor_copy(B_vw[:, y0:y1, 63], z64[:, 0:y1 - y0])

            # ---- background work, split so dependency chains lag the
            # ---- in-order engine streams by a pair or more ----------------
            def bg_front(t):
                # conv3x3 tile + BN/relu + edge 1x1 (PE + DVE)
                bg_prep(t)
                t0 = 512 * t
                ep_ps = ps_bg.tile([CH, 512], F32, tag="bg")
                mms = []
                for dy in (0, -1, 1):
                    lo = max(t0, -64 * dy)
                    hi = min(t0 + 512, HW - max(0, 64 * dy))
                    if hi <= lo:
                        continue
                    sl_out = ep_ps[:, lo - t0:hi - t0]
                    ky = dy + 1
                    mms.append((sl_out, wcatA_r[:, 32 * ky:32 * ky + 32],
                                A[:, lo + 64 * dy:hi + 64 * dy]))
                    mms.append((sl_out, wcatB_r[:, 32 * ky:32 * ky + 32],
                                B[0:64, lo + 64 * dy:hi + 64 * dy]))
                for i, (o, l, r) in enumerate(mms):
                    nc.tensor.matmul(o, l, r, start=(i == 0),
                                     stop=(i == len(mms) - 1),
                                     skip_group_check=True)
                nc.vector.tensor_scalar(
                    out=relu_sb[:, t0:t0 + 512], in0=ep_ps[:],
                    scalar1=bnt_sb[:, 0:1], scalar2=0.0,
                    op0=ALU.add, op1=ALU.max)
                eg_t = ps_bg.tile([128, 8], F32, tag="bg")
                for jj in range(4):
                    j = 4 * t + jj
                    nc.tensor.matmul(eg_t[:, 2 * jj:2 * jj + 2],
                                     relu_sb[:, 128 * j:128 * j + 128],
                                     we2_r[:], start=True, stop=True,
                                     skip_group_check=True)
                return eg_t

            # sigmoid(z) ~= 0.5 + z*g(z^2) on [-5,5]; computed DVE-only so
            # the ACT stream stays pure exp.  beta is folded into g.
            SIGC = [-3.4531099160e-08, 2.7521357982e-06, -8.9630342335e-05, 1.6215920842e-03, -2.0098424428e-02, 2.4979733221e-01]

            def bg_back(t, eg_t):
                g0 = 8 * t
                if CFG["sigma"] == "act":
                    nc.scalar.activation(eg_sb[:, g0:g0 + 8], eg_t[:], AF.Exp,
                                         bias=b_be2[:], scale=-1.0)
                    nc.vector.tensor_scalar_add(eg_sb[:, g0:g0 + 8],
                                                eg_sb[:, g0:g0 + 8], 1.0)
                    nc.vector.reciprocal(eg_sb[:, g0:g0 + 8],
                                         eg_sb[:, g0:g0 + 8])
                    nc.vector.tensor_scalar(out=modt[:, g0:g0 + 8],
                                            in0=eg_sb[:, g0:g0 + 8],
                                            scalar1=beta, scalar2=1.0,
                                            op0=ALU.mult, op1=ALU.add)
                    nc.vector.tensor_copy(modr[:, g0:g0 + 8],
                                          modt[:, g0:g0 + 8])
                    modveccopy(t)
                    vt_mms(t)
                    return
                u = eg_sb[:, g0:g0 + 8]
                nc.vector.tensor_scalar(out=u, in0=eg_t[:], scalar1=be2,
                                        scalar2=-5.0, op0=ALU.add, op1=ALU.max)
                nc.vector.tensor_scalar_min(u, u, 5.0)
                w = modt[:, g0:g0 + 8]  # scratch, becomes mod at the end
                nc.vector.tensor_mul(w, u, u)
                g = eg2_sb[:, g0:g0 + 8]
                nc.vector.tensor_scalar(out=g, in0=w, scalar1=beta * SIGC[0],
                                        scalar2=beta * SIGC[1], op0=ALU.mult,
                                        op1=ALU.add)
                for ck in SIGC[2:]:
                    nc.vector.tensor_mul(g, g, w)
                    nc.vector.tensor_scalar_add(g, g, beta * ck)
                nc.vector.tensor_mul(g, g, u)
                nc.vector.tensor_scalar_add(modt[:, g0:g0 + 8], g,
                                            1.0 + 0.5 * beta)
                nc.vector.tensor_copy(modr[:, g0:g0 + 8], modt[:, g0:g0 + 8])
                modveccopy(t)
                vt_mms(t)

            def modveccopy(t):
                nc.vector.tensor_copy(VT_v[:, 4 * t:4 * t + 4, 64],
                                      modr_v[:, 4 * t:4 * t + 4, 0])

            def vt_mms(t):
                for jj in (0, 2):
                    j = 4 * t + jj
                    vpA = ps_bg.tile([128, 64], F32, tag="bg")
                    vpB = ps_bg.tile([128, 64], F32, tag="bg")
                    nc.tensor.matmul(vpA[:], x_r[:, 128 * j:128 * j + 128],
                                     wv_r[0:64, :], start=True, stop=True)
                    nc.tensor.matmul(vpB[:],
                                     A[64:128, 128 * (j + 1):128 * (j + 1) + 128],
                                     wv_r[64:128, :], start=True, stop=True)
                    for jx, vp in ((j, vpA), (j + 1, vpB)):
                        nc.vector.tensor_scalar_mul(
                            VT[:, 66 * jx:66 * jx + 64], vp[:],
                            modt[:, 2 * jx:2 * jx + 1])

            # ---- main attention loop (row-packed QK chunk pairs).
            # Per position: QK+exp for pair p, conv front for tile p,
            # sigma/V\' back for tile p-1, and AV for pair p-LAG.
            def emit_qk_exp(qb, pair, fa_store):
                q0 = QB * qb
                mc0, mc1 = 2 * pair, 2 * pair + 1
                sA = ps.tile([128, QB], F32, tag="ps")
                sB = ps.tile([128, QB], F32, tag="ps")
                for h in range(QB // 512):
                    nc.tensor.matmul(
                        sA[:, 512 * h:512 * h + 512],
                        K2[0:64, 128 * mc0:128 * mc0 + 128],
                        Q2[0:64, q0 + 512 * h:q0 + 512 * h + 512],
                        start=True, stop=True)
                    nc.tensor.matmul(
                        sB[:, 512 * h:512 * h + 512],
                        K2[64:128, 128 * mc1:128 * mc1 + 128],
                        Q2[64:128, q0 + 512 * h:q0 + 512 * h + 512],
                        start=True, stop=True)
                for s_ps in (sA, sB):
                    fa = fa_p.tile([128, QB], F32R)
                    nc.scalar.activation(fa[:], s_ps[:], AF.Exp,
                                         bias=b_shift[:], scale=1.0)
                    fa_store.append(fa)

            def emit_av(o_ps, pair, gpair, fa_store):
                for k, mc in enumerate((2 * pair, 2 * pair + 1)):
                    fa = fa_store[2 * gpair + k]
                    for h in range(QB // 512):
                        nc.tensor.matmul(
                            o_ps[:, 512 * h:512 * h + 512],
                            VT[:, 66 * mc:66 * mc + 66],
                            fa[:, 512 * h:512 * h + 512],
                            start=(mc == 0), stop=(mc == 31),
                            skip_group_check=True)
                    fa_store[2 * gpair + k] = None

            NPAIR = 16
            emit_kproj(0)
            emit_qproj(0)
            emit_qproj(1)
            kproj_done = 1
            qproj_done = 2

            def epilogue(qb, o_ps):
                # out = O' * (gamma/denom) + xq ; denom is o_ps row 64
                rc = ep.tile([65, QB], F32)
                nc.vector.reciprocal(rc[64:65, :], o_ps[64:65, :])
                r2 = ep.tile([65, QB], F32R)
                nc.vector.tensor_copy(r2[64:65, :], rc[64:65, :])
                R_sb = ep.tile([64, QB], F32)
                o2 = ep.tile([64, QB], F32)
                for h in range(QB // 512):
                    sl = slice(512 * h, 512 * h + 512)
                    r_ps = ps_bg.tile([64, 512], F32, tag="bg")
                    nc.tensor.matmul(r_ps[:], gam_r[64:65, 0:64],
                                     r2[64:65, sl], start=True, stop=True)
                    nc.vector.tensor_copy(R_sb[:, sl], r_ps[:])
                    nc.vector.tensor_mul(R_sb[:, sl], o_ps[0:64, sl], R_sb[:, sl])
                    nc.vector.tensor_add(o2[:, sl], R_sb[:, sl],
                                         xq_f[:, QB * qb + 512 * h:QB * qb + 512 * h + 512])
                    nc.sync.dma_start(out_d[:, QB * qb + 512 * h:QB * qb + 512 * h + 512],
                                      o2[:, sl])

            fa_store = []
            o_tiles = {}
            av_next = 0
            bg_done = 0
            eg_prev = None
            NPOS = 2 * NPAIR
            for pos in range(NPOS + 2):
                if pos < NPOS:
                    qb, pair = pos // NPAIR, pos % NPAIR
                    emit_qk_exp(qb, pair, fa_store)
                # K/Q projections stay ahead of the QK pairs
                while kproj_done < min(pos // 2 + 2, HW // 512):
                    emit_kproj(kproj_done)
                    kproj_done += 1
                if pos == 2 and qproj_done < 4:
                    emit_qproj(2)
                    emit_qproj(3)
                    qproj_done = 4
                if pos == 0:
                    nc.sync.dma_start(out=bnt_sb[:], in_=bnt_d[:])
                    emit_xprep()
                if pos == 12:
                    nc.sync.dma_start(out=xq_f[:], in_=xq_d[:])
                if eg_prev is not None:
                    bg_back(bg_done - 1, eg_prev)
                    eg_prev = None
                if bg_done < 8 and pos >= CFG["bg_start"] and pos not in CFG["bg_skips"]:
                    eg_prev = bg_front(bg_done)
                    bg_done += 1
                # AV drains with a lag that shrinks once bg work is done
                lag = CFG["lag_hi"] if pos < CFG["lag_switch"] else CFG["lag_lo"]
                quota = 2
                while av_next <= pos - lag and av_next < NPOS and quota > 0:
                    aqb, apair = av_next // NPAIR, av_next % NPAIR
                    if apair == 0:
                        o_tiles[aqb] = ps_o.tile([66, QB], F32, tag="o",
                                                 name=f"o_ps_{aqb}")
                    emit_av(o_tiles[aqb], apair, av_next, fa_store)
                    quota -= 1
                    av_next += 1
                    if apair == NPAIR - 1:
                        epilogue(aqb, o_tiles[aqb])

    nc.compile()
    return nc


def prep_inputs(inputs: dict):
    """Host-side preprocessing: returns (in_maps, scalars, out shape info)."""
    x = np.asarray(inputs["x"], np.float32)        # (B, C, H, W)
    Bsz = x.shape[0]
    Wq = np.asarray(inputs["Wq"], np.float32)
    Wk = np.asarray(inputs["Wk"], np.float32)
    Wv = np.asarray(inputs["Wv"], np.float32)
    We1 = np.asarray(inputs["We1"], np.float32)    # (CH, C, 3, 3)
    be1 = np.asarray(inputs["be1"], np.float32)
    bn_w = np.asarray(inputs["bn_w"], np.float32)
    bn_b = np.asarray(inputs["bn_b"], np.float32)
    bn_mean = np.asarray(inputs["bn_mean"], np.float32)
    bn_var = np.asarray(inputs["bn_var"], np.float32)
    We2 = np.asarray(inputs["We2"], np.float32)    # (1, CH)
    be2 = float(np.asarray(inputs["be2"]).reshape(-1)[0])
    gamma = float(np.asarray(inputs["gamma"]).reshape(-1)[0])
    beta = float(np.asarray(inputs["beta"]).reshape(-1)[0])
    assert abs(beta) < 0.999, "kernel assumes 1 + beta*edge > 0"

    bn_s = bn_w / np.sqrt(bn_var + BN_EPS)
    We1s = We1 * bn_s[:, None, None, None]
    bn_t = (be1 - bn_mean) * bn_s + bn_b

    # A rows 0-63 hold x shifted so col f = x[f-1] (left neighbor, kx=0);
    # A rows 64-127 hold x itself (kx=1); B holds x[f+1] (right, kx=2).
    wcatA = np.zeros((128, 96), np.float32)
    wcatB = np.zeros((64, 96), np.float32)
    for ky in range(3):
        wcatA[0:64, 32 * ky:32 * ky + 32] = We1s[:, :, ky, 0].T
        wcatA[64:128, 32 * ky:32 * ky + 32] = We1s[:, :, ky, 1].T
        wcatB[0:64, 32 * ky:32 * ky + 32] = We1s[:, :, ky, 2].T

    we2_t = np.repeat(We2.reshape(1, CH).T, 2, axis=1)  # duplicated column

    shared = {
        "wq_t": np.ascontiguousarray(Wq.T),
        "wk_t": np.ascontiguousarray(Wk.T),
        "wv_t": np.ascontiguousarray(Wv.T),
        "wcatA": wcatA,
        "wcatB": wcatB,
        "we2_t": np.ascontiguousarray(we2_t),
        "bn_t": bn_t.reshape(CH, 1),
    }
    in_maps = []
    for core in range(N_CORES):
        b, h = core // 2, core % 2
        xb = np.ascontiguousarray(x[b].reshape(C, HW))
        m = dict(shared)
        m["x"] = xb
        m["xq"] = np.ascontiguousarray(xb[:, h * NQ:(h + 1) * NQ])
        in_maps.append(m)
    return in_maps, (beta, gamma, be2), (Bsz, x.shape[2], x.shape[3])


_cache = {}


def get_program(scalars):
    if scalars not in _cache:
        _cache[scalars] = build_program(*scalars)
    return _cache[scalars]


def kernel(**inputs) -> np.ndarray:
    in_maps, scalars, (Bsz, H, W) = prep_inputs(inputs)
    nc = get_program(scalars)
    res = run_bass_kernel_spmd(nc, in_maps, list(range(N_CORES)))
    out = np.empty((Bsz, C, H * W), np.float32)
    for core in range(N_CORES):
        b, h = core // 2, core % 2
        out[b][:, h * NQ:(h + 1) * NQ] = res.results[core]["out"]
    return out.reshape(Bsz, C, H, W)

